# revision 9
# baseline (speedup 1.0000x reference)
"""TRN2 Bass kernel for nn_CIFAR10_Type1_Template_Unroll (dense_cnn).

Network (per reference): two locally-connected conv layers + 3-layer FC
head, B=4096, all fp32. Strategy: pure data parallel over 8 NeuronCores
(512 batch each), activations kept on-chip in [feature, batch] layout.
Matmuls run in fp32r (full PE rate for N>=256, ~1e-4 relative error)
except L2, which runs in fp16 (~5e-4) so pairs of output positions can
execute concurrently on the PE's column strips (tile_position col
tiling is rejected by walrus for 4-byte dtypes).

Layer mapping per core (batch N=512 on the matmul free dim throughout):
- L1 (k=2,s=2 locally-connected): patches are a pure reshape of x. Host
  packs, per output row r and pair of adjacent positions, a K=32 strip
  (2 positions x 16 feats: 12 real + 4 zero-pad) and a block-diagonal
  [32, 128] weight tile. 4 pairs run concurrently on the PE's 32-row
  strips via tile_position=(32i, 0).
- L2 (k=4,s=2): positions are paired (matching the h2 tile halves);
  the two members run concurrently on col strips 0-63 / 64-127 via
  tile_position (0,0)/(0,64), each accumulating 8 K-tile matmuls
  [128K, 64M] in its own PSUM bank (start=True clears a whole bank, so
  chains must not share one).
- FC head: standard K/M tiling; FC weights host-permuted to match the
  on-chip feature order of h2 ([pos-pair, parity, channel]).
Host-side prep only reshapes/permutes weights and input (numpy).
"""
import sys

if '/opt/trn_rl_repo' not in sys.path:
    sys.path.insert(0, '/opt/trn_rl_repo')

import numpy as np

N_CORES = 8
BS = 512
LAST_EXEC_NS = None

# ----------------------------------------------------------------- host prep

def _prep_x(x):
    """x [B,3,32,32] -> [N_CORES, 16, 2, 128, 512] patch tiles.

    part = 32*i + 16*q + f; pair p=4g+i covers w1 in {2p,2p+1}; q = w1
    parity; f = c*4 + kh*2 + kw (12..15 zero-pad). Free dim = batch.
    """
    ncr = x.shape[0] // BS
    xr = x.reshape(ncr, BS, 3, 16, 2, 2, 4, 2, 2)   # s,b,c,r,kh,g,i,q,kw
    xt = xr.transpose(0, 3, 5, 6, 7, 2, 4, 8, 1)    # s,r,g,i,q,c,kh,kw,b
    xt = xt.reshape(ncr, 16, 2, 4, 2, 12, BS)
    xpp = np.zeros((ncr, 16, 2, 4, 2, 16, BS), np.float32)
    xpp[..., :12, :] = xt
    return np.ascontiguousarray(xpp.reshape(ncr, 16, 2, 128, BS))


def _prep_w1(conv1w):
    """conv1w [64,256,3,2,2] -> [16, 128, 2, 128] block-diag strips."""
    w1r = conv1w.reshape(64, 16, 16, 3, 2, 2)
    wt = w1r.transpose(1, 2, 3, 4, 5, 0).reshape(16, 16, 12, 64)
    wtp = np.zeros((16, 16, 16, 64), np.float32)
    wtp[:, :, :12, :] = wt
    wtp = wtp.reshape(16, 2, 4, 2, 16, 64)          # r,g,i,qp,f,o
    w1t = np.zeros((16, 2, 4, 2, 16, 2, 64), np.float32)
    w1t[:, :, :, 0, :, 0, :] = wtp[:, :, :, 0, :, :]
    w1t[:, :, :, 1, :, 1, :] = wtp[:, :, :, 1, :, :]
    w1t = w1t.reshape(16, 2, 128, 128).transpose(0, 2, 1, 3)
    return np.ascontiguousarray(w1t)


def _prep_w2(conv2w):
    """conv2w [64,49,64,4,4] -> [49, 128, 512] (fp16)."""
    w2r = conv2w.reshape(64, 7, 7, 64, 4, 4)
    v = w2r.transpose(1, 2, 3, 4, 5, 0)             # h,w,c,kh,kw,o
    v = v.reshape(7, 7, 64, 4, 2, 2, 64)            # h,w,c,kh,t,q,o
    v = v.transpose(0, 1, 5, 2, 3, 4, 6)            # h,w,q,c,kh,t,o
    return np.ascontiguousarray(v.reshape(49, 128, 512)).astype(np.float16)


def _h2_posmap():
    pm = np.full((25, 2), -1, np.int64)
    for T in range(21):
        rr, j = divmod(T, 3)
        pm[T, 0] = rr * 7 + 2 * j
        pm[T, 1] = rr * 7 + 2 * j + 1
    for pi in range(4):
        r0, r1 = 2 * pi, 2 * pi + 1
        pm[21 + pi, 0] = r0 * 7 + 6
        if r1 < 7:
            pm[21 + pi, 1] = r1 * 7 + 6
    return pm


def _prep_fc1(fc1):
    pm = _h2_posmap()
    fc1p = fc1.reshape(1024, 64, 49)
    fc1hat = np.zeros((1024, 25, 2, 64), np.float32)
    for T in range(25):
        for u in range(2):
            p = pm[T, u]
            if p >= 0:
                fc1hat[:, T, u, :] = fc1p[:, :, p]
    a = fc1hat.reshape(1024, 25, 128).reshape(8, 128, 25, 128)
    return np.ascontiguousarray(a.transpose(0, 3, 2, 1))   # m,kp,k,mc


def _prep_fc2(fc2):
    a = fc2.reshape(4, 128, 8, 128)
    return np.ascontiguousarray(a.transpose(0, 3, 2, 1))   # m,kp,k,mc


def _prep_fc3(fc3):
    a = fc3.T.reshape(4, 128, 10)
    return np.ascontiguousarray(a.transpose(1, 0, 2))      # kp,k,o


# --------------------------------------------------------------- bass kernel

_NC_CACHE = []


def _build_nc():
    import concourse.bass as bass
    import concourse.mybir as mybir
    from concourse import bacc
    from concourse.tile import TileContext

    f32 = mybir.dt.float32
    f32r = mybir.dt.float32r
    f16 = mybir.dt.float16
    RELU = mybir.ActivationFunctionType.Relu
    rc = lambda ap: ap

    nc = bacc.Bacc("TRN2", target_bir_lowering=False, debug=False,
                   num_devices=N_CORES)
    x_pp = nc.dram_tensor("x_pp", [16, 2, 128, BS], f32r, kind="ExternalInput")
    w1t = nc.dram_tensor("w1t", [16, 128, 2, 128], f32r, kind="ExternalInput")
    w2t = nc.dram_tensor("w2t", [49, 128, 512], f16, kind="ExternalInput")
    fc1m = nc.dram_tensor("fc1m", [8, 128, 25, 128], f32r, kind="ExternalInput")
    fc2t = nc.dram_tensor("fc2t", [4, 128, 8, 128], f32r, kind="ExternalInput")
    fc3t = nc.dram_tensor("fc3t", [128, 4, 10], f32r, kind="ExternalInput")
    zeros64 = nc.dram_tensor("zeros64", [64, 512], f32r, kind="ExternalInput")
    y = nc.dram_tensor("y", [BS, 10], f32, kind="ExternalOutput")

    pm = _h2_posmap()
    tile_of_pos = {}
    for T in range(25):
        for u in range(2):
            if pm[T, u] >= 0:
                tile_of_pos[pm[T, u]] = (T, u)

    ectr = [0]

    with TileContext(nc) as tc:
        def relu_evac(dst, src):
            if ectr[0] % 2 == 0:
                nc.scalar.activation(dst, src, RELU)
            else:
                nc.vector.tensor_scalar_max(dst, src, 0.0)
            ectr[0] += 1

        with (
            tc.tile_pool(name="h2pool", bufs=25) as h2pool,
            tc.tile_pool(name="fcw", bufs=2) as fcw_pool,
        ):
            h2 = [h2pool.tile([128, 512], f32r, tag="h2", name=f"h2_{T}")
                  for T in range(25)]
            # --------------- phase 1: L1 + L2 interleaved ---------------
            with (
                tc.tile_pool(name="xp", bufs=4) as xpp_pool,
                tc.tile_pool(name="w1p", bufs=3) as w1_pool,
                tc.tile_pool(name="w2p", bufs=6) as w2_pool,
                tc.tile_pool(name="o1p", bufs=64) as o1_pool,
                tc.tile_pool(name="l1ps", bufs=4, space="PSUM") as l1ps,
                tc.tile_pool(name="l2ps", bufs=4, space="PSUM") as l2ps,
            ):
                nc.sync.dma_start(out=h2[24][64:128, :], in_=zeros64.ap()[:])
                # PE warmup: keep the array busy during the initial DMA
                # ramp so HAM un-throttles before real matmuls arrive.
                # Dummy MMs over the (already zeroed) h2[24] hi half; the
                # consumed psum bank is start=True-cleared by later users.
                wps = l1ps.tile([128, 512], f32, tag="l1", name="warm_ps")
                zsrc = h2[24][64:96, 0:512]
                for wi in range(20):
                    nc.tensor.matmul(wps[:], zsrc[:, 0:128], zsrc[:, :],
                                     start=True, stop=True)
                out1 = [[None] * 8 for _ in range(16)]

                def emit_l1_row(r):
                    w1row = w1_pool.tile([128, 256], f32r, tag="w1",
                                         name=f"w1_{r}")
                    nc.sync.dma_start(
                        out=w1row[:],
                        in_=w1t.ap()[r].rearrange("p g c -> p (g c)"))
                    for g in range(2):
                        xt = xpp_pool.tile([128, BS], f32r, tag="xp",
                                           name=f"xp_{r}_{g}")
                        nc.sync.dma_start(out=xt[:], in_=x_pp.ap()[r, g])
                        for i in range(4):
                            ps = l1ps.tile([128, 512], f32, tag="l1",
                                           name=f"l1ps_{r}_{g}_{i}")
                            nc.tensor.matmul(
                                ps[:],
                                rc(w1row[32*i:32*i+32, 128*g:128*g+128]),
                                rc(xt[32*i:32*i+32, :]),
                                start=True, stop=True,
                                tile_position=(32 * i, 0))
                            ot = o1_pool.tile([128, 512], f16, tag="o1",
                                              name=f"o1_{r}_{4*g+i}")
                            relu_evac(ot[:], ps[:])
                            out1[r][4 * g + i] = ot

                def load_w2(pos):
                    w2til = w2_pool.tile([128, 512], f16, tag="w2",
                                         name=f"w2_{pos}")
                    nc.sync.dma_start(out=w2til[:], in_=w2t.ap()[pos])
                    return w2til

                def emit_l2_pair(T, hA, wA, hB, wB):
                    # Two positions concurrently on PE col strips 0-63 /
                    # 64-127 (tile_position col tiling), each chain
                    # accumulating in its own PSUM bank so the start=True
                    # bank clears stay independent of scheduler order.
                    wtA = load_w2(hA * 7 + wA)
                    wtB = None if hB is None else load_w2(hB * 7 + wB)
                    psA = l2ps.tile([128, 512], f32, tag="l2",
                                    name=f"l2psA_{T}")
                    psB = None
                    if wtB is not None:
                        psB = l2ps.tile([128, 512], f32, tag="l2",
                                        name=f"l2psB_{T}")
                    for kt in range(8):
                        kh, t = divmod(kt, 2)
                        nc.tensor.matmul(
                            psA[0:64, :],
                            wtA[:, 64*kt:64*kt+64],
                            out1[2*hA+kh][wA+t][:],
                            start=(kt == 0), stop=(kt == 7),
                            tile_position=(0, 0))
                        if wtB is not None:
                            nc.tensor.matmul(
                                psB[64:128, :],
                                wtB[:, 64*kt:64*kt+64],
                                out1[2*hB+kh][wB+t][:],
                                start=(kt == 0), stop=(kt == 7),
                                tile_position=(0, 64))
                    relu_evac(h2[T][0:64, :], psA[0:64, :])
                    if wtB is not None:
                        relu_evac(h2[T][64:128, :], psB[64:128, :])

                def emit_l2_pass(h):
                    for j in range(3):
                        emit_l2_pair(h * 3 + j, h, 2 * j, h, 2 * j + 1)
                    if h % 2 == 1:
                        pi = (h - 1) // 2
                        emit_l2_pair(21 + pi, h - 1, 6, h, 6)
                    if h == 6:
                        emit_l2_pair(24, 6, 6, None, None)

                for r in range(16):
                    emit_l1_row(r)
                    if r >= 3 and r % 2 == 1:
                        emit_l2_pass((r - 3) // 2)

            # --------------- phase 2: FC head ---------------
            with (
                tc.tile_pool(name="fcio", bufs=12) as fcio_pool,
                tc.tile_pool(name="fcps", bufs=2, space="PSUM") as fcps,
                tc.tile_pool(name="fc3ps", bufs=2, space="PSUM") as fc3ps,
            ):
                h3 = []
                for m in range(8):
                    wt = fcw_pool.tile([128, 25 * 128], f32r, tag="fc1w",
                                       name=f"fc1w_{m}")
                    src = fc1m.ap()[m].rearrange("p k c -> p (k c)")
                    nc.sync.dma_start(out=wt[:, 0:1600], in_=src[:, 0:1600])
                    nc.sync.dma_start(out=wt[:, 1600:3200],
                                      in_=src[:, 1600:3200])
                    ps = fcps.tile([128, 512], f32, tag="fc",
                                   name=f"fc1ps_{m}")
                    for k in range(25):
                        nc.tensor.matmul(ps[:],
                                         rc(wt[:, 128*k:128*k+128]),
                                         rc(h2[k][:]),
                                         start=(k == 0), stop=(k == 24))
                    ot = fcio_pool.tile([128, 512], f32r, tag="h3",
                                        name=f"h3_{m}", bufs=8)
                    relu_evac(ot[:], ps[:])
                    h3.append(ot)
                h4 = []
                for m in range(4):
                    wt = fcw_pool.tile([128, 8 * 128], f32r, tag="fc2w",
                                       name=f"fc2w_{m}")
                    nc.sync.dma_start(
                        out=wt[:],
                        in_=fc2t.ap()[m].rearrange("p k c -> p (k c)"))
                    ps = fcps.tile([128, 512], f32, tag="fc",
                                   name=f"fc2ps_{m}")
                    for k in range(8):
                        nc.tensor.matmul(ps[:],
                                         rc(wt[:, 128*k:128*k+128]),
                                         rc(h3[k][:]),
                                         start=(k == 0), stop=(k == 7))
                    ot = fcio_pool.tile([128, 512], f32r, tag="h4",
                                        name=f"h4_{m}", bufs=4)
                    relu_evac(ot[:], ps[:])
                    h4.append(ot)
                w3 = fcio_pool.tile([128, 40], f32r, tag="fc3w",
                                    name="fc3w", bufs=1)
                nc.sync.dma_start(
                    out=w3[:], in_=fc3t.ap().rearrange("p k o -> p (k o)"))
                for b4 in range(4):
                    ps = fc3ps.tile([128, 10], f32, tag="fc3",
                                    name=f"fc3ps_{b4}")
                    for k in range(4):
                        nc.tensor.matmul(
                            ps[:],
                            rc(h4[k][:, 128*b4:128*b4+128]),
                            rc(w3[:, 10*k:10*k+10]),
                            start=(k == 0), stop=(k == 3))
                    ot = fcio_pool.tile([128, 10], f32, tag="yout",
                                        name=f"y_{b4}", bufs=4)
                    nc.vector.tensor_copy(ot[:], ps[:])
                    nc.sync.dma_start(out=y.ap()[128*b4:128*b4+128, :],
                                      in_=ot[:])
    nc.compile()
    return nc


def kernel(x, conv1w, conv2w, fc1, fc2, fc3):
    global LAST_EXEC_NS
    from concourse.bass_utils import run_bass_kernel_spmd

    x = np.ascontiguousarray(np.asarray(x, dtype=np.float32))
    conv1w = np.ascontiguousarray(np.asarray(conv1w, dtype=np.float32))
    conv2w = np.ascontiguousarray(np.asarray(conv2w, dtype=np.float32))
    fc1 = np.ascontiguousarray(np.asarray(fc1, dtype=np.float32))
    fc2 = np.ascontiguousarray(np.asarray(fc2, dtype=np.float32))
    fc3 = np.ascontiguousarray(np.asarray(fc3, dtype=np.float32))

    if not _NC_CACHE:
        _NC_CACHE.append(_build_nc())
    nc = _NC_CACHE[0]

    xpp = _prep_x(x)
    shared = {
        "zeros64": np.zeros((64, 512), np.float32),
        "w1t": _prep_w1(conv1w),
        "w2t": _prep_w2(conv2w),
        "fc1m": _prep_fc1(fc1),
        "fc2t": _prep_fc2(fc2),
        "fc3t": _prep_fc3(fc3),
    }
    in_maps = [{**shared, "x_pp": xpp[c]} for c in range(N_CORES)]
    res = run_bass_kernel_spmd(nc, in_maps, list(range(N_CORES)))
    LAST_EXEC_NS = res.exec_time_ns
    return np.concatenate([r["y"] for r in res.results], axis=0)


# revision 15
# speedup vs baseline: 1.0428x; 1.0428x over previous
"""TRN2 Bass kernel for nn_CIFAR10_Type1_Template_Unroll (dense_cnn).

Network (per reference): two locally-connected conv layers + 3-layer FC
head, B=4096, all fp32. Strategy: pure data parallel over 8 NeuronCores
(512 batch each), activations kept on-chip in [feature, batch] layout.
Matmuls run in fp32r (full PE rate for N>=256, ~1e-4 relative error)
except L2, which runs in fp16 (~5e-4) so pairs of output positions can
execute concurrently on the PE's column strips (tile_position col
tiling is rejected by walrus for 4-byte dtypes).

Layer mapping per core (batch N=512 on the matmul free dim throughout):
- L1 (k=2,s=2 locally-connected): patches are a pure reshape of x. Host
  packs, per output row r and pair of adjacent positions, a K=32 strip
  (2 positions x 16 feats: 12 real + 4 zero-pad) and a block-diagonal
  [32, 128] weight tile. 4 pairs run concurrently on the PE's 32-row
  strips via tile_position=(32i, 0).
- L2 (k=4,s=2): positions are paired (matching the h2 tile halves);
  the two members run concurrently on col strips 0-63 / 64-127 via
  tile_position (0,0)/(0,64), each accumulating 8 K-tile matmuls
  [128K, 64M] in its own PSUM bank (start=True clears a whole bank, so
  chains must not share one).
- FC head: standard K/M tiling; FC weights host-permuted to match the
  on-chip feature order of h2 ([pos-pair, parity, channel]).
Host-side prep only reshapes/permutes weights and input (numpy).
"""
import sys

if '/opt/trn_rl_repo' not in sys.path:
    sys.path.insert(0, '/opt/trn_rl_repo')

import numpy as np

N_CORES = 8
BS = 512
LAST_EXEC_NS = None

# ----------------------------------------------------------------- host prep

def _prep_x(x):
    """x [B,3,32,32] -> [N_CORES, 16, 2, 128, 512] patch tiles.

    part = 32*i + 16*q + f; pair p=4g+i covers w1 in {2p,2p+1}; q = w1
    parity; f = c*4 + kh*2 + kw (12..15 zero-pad). Free dim = batch.
    """
    ncr = x.shape[0] // BS
    xr = x.reshape(ncr, BS, 3, 16, 2, 2, 4, 2, 2)   # s,b,c,r,kh,g,i,q,kw
    xt = xr.transpose(0, 3, 5, 6, 7, 2, 4, 8, 1)    # s,r,g,i,q,c,kh,kw,b
    xt = xt.reshape(ncr, 16, 2, 4, 2, 12, BS)
    xpp = np.zeros((ncr, 16, 2, 4, 2, 16, BS), np.float32)
    xpp[..., :12, :] = xt
    return np.ascontiguousarray(xpp.reshape(ncr, 16, 2, 128, BS))


def _prep_w1(conv1w):
    """conv1w [64,256,3,2,2] -> [16, 128, 2, 128] block-diag strips."""
    w1r = conv1w.reshape(64, 16, 16, 3, 2, 2)
    wt = w1r.transpose(1, 2, 3, 4, 5, 0).reshape(16, 16, 12, 64)
    wtp = np.zeros((16, 16, 16, 64), np.float32)
    wtp[:, :, :12, :] = wt
    wtp = wtp.reshape(16, 2, 4, 2, 16, 64)          # r,g,i,qp,f,o
    w1t = np.zeros((16, 2, 4, 2, 16, 2, 64), np.float32)
    w1t[:, :, :, 0, :, 0, :] = wtp[:, :, :, 0, :, :]
    w1t[:, :, :, 1, :, 1, :] = wtp[:, :, :, 1, :, :]
    w1t = w1t.reshape(16, 2, 128, 128).transpose(0, 2, 1, 3)
    return np.ascontiguousarray(w1t)


def _prep_w2(conv2w):
    """conv2w [64,49,64,4,4] -> [49, 128, 512] (fp16)."""
    w2r = conv2w.reshape(64, 7, 7, 64, 4, 4)
    v = w2r.transpose(1, 2, 3, 4, 5, 0)             # h,w,c,kh,kw,o
    v = v.reshape(7, 7, 64, 4, 2, 2, 64)            # h,w,c,kh,t,q,o
    v = v.transpose(0, 1, 5, 2, 3, 4, 6)            # h,w,q,c,kh,t,o
    return np.ascontiguousarray(v.reshape(49, 128, 512)).astype(np.float16)


def _h2_posmap():
    pm = np.full((25, 2), -1, np.int64)
    for T in range(21):
        rr, j = divmod(T, 3)
        pm[T, 0] = rr * 7 + 2 * j
        pm[T, 1] = rr * 7 + 2 * j + 1
    for pi in range(4):
        r0, r1 = 2 * pi, 2 * pi + 1
        pm[21 + pi, 0] = r0 * 7 + 6
        if r1 < 7:
            pm[21 + pi, 1] = r1 * 7 + 6
    return pm


def _prep_fc1(fc1):
    pm = _h2_posmap()
    fc1p = fc1.reshape(1024, 64, 49)
    fc1hat = np.zeros((1024, 25, 2, 64), np.float32)
    for T in range(25):
        for u in range(2):
            p = pm[T, u]
            if p >= 0:
                fc1hat[:, T, u, :] = fc1p[:, :, p]
    a = fc1hat.reshape(1024, 25, 128).reshape(8, 128, 25, 128)
    return np.ascontiguousarray(a.transpose(0, 3, 2, 1))   # m,kp,k,mc


def _prep_fc2(fc2):
    a = fc2.reshape(4, 128, 8, 128)
    return np.ascontiguousarray(a.transpose(0, 3, 2, 1))   # m,kp,k,mc


def _prep_fc3(fc3):
    a = fc3.T.reshape(4, 128, 10)
    return np.ascontiguousarray(a.transpose(1, 0, 2))      # kp,k,o


# --------------------------------------------------------------- bass kernel

_NC_CACHE = []


def _build_nc():
    import concourse.bass as bass
    import concourse.mybir as mybir
    from concourse import bacc
    from concourse.tile import TileContext

    f32 = mybir.dt.float32
    f32r = mybir.dt.float32r
    f16 = mybir.dt.float16
    RELU = mybir.ActivationFunctionType.Relu
    rc = lambda ap: ap

    nc = bacc.Bacc("TRN2", target_bir_lowering=False, debug=False,
                   num_devices=N_CORES)
    x_pp = nc.dram_tensor("x_pp", [16, 2, 128, BS], f32r, kind="ExternalInput")
    w1t = nc.dram_tensor("w1t", [16, 128, 2, 128], f32r, kind="ExternalInput")
    w2t = nc.dram_tensor("w2t", [49, 128, 512], f16, kind="ExternalInput")
    fc1m = nc.dram_tensor("fc1m", [8, 128, 25, 128], f32r, kind="ExternalInput")
    fc2t = nc.dram_tensor("fc2t", [4, 128, 8, 128], f32r, kind="ExternalInput")
    fc3t = nc.dram_tensor("fc3t", [128, 4, 10], f32r, kind="ExternalInput")
    zeros64 = nc.dram_tensor("zeros64", [64, 512], f32r, kind="ExternalInput")
    y = nc.dram_tensor("y", [BS, 10], f32, kind="ExternalOutput")

    pm = _h2_posmap()
    tile_of_pos = {}
    for T in range(25):
        for u in range(2):
            if pm[T, u] >= 0:
                tile_of_pos[pm[T, u]] = (T, u)

    ectr = [0]

    with TileContext(nc) as tc:
        def relu_evac(dst, src):
            if ectr[0] % 2 == 0:
                nc.scalar.activation(dst, src, RELU)
            else:
                nc.vector.tensor_scalar_max(dst, src, 0.0)
            ectr[0] += 1

        with (
            tc.tile_pool(name="h2pool", bufs=25) as h2pool,
            tc.tile_pool(name="fcw", bufs=2) as fcw_pool,
        ):
            h2 = [h2pool.tile([128, 512], f32r, tag="h2", name=f"h2_{T}")
                  for T in range(25)]
            # --------------- phase 1: L1 + L2 interleaved ---------------
            with (
                tc.tile_pool(name="xp", bufs=4) as xpp_pool,
                tc.tile_pool(name="w1p", bufs=3) as w1_pool,
                tc.tile_pool(name="w2p", bufs=6) as w2_pool,
                tc.tile_pool(name="o1p", bufs=72) as o1_pool,
                tc.tile_pool(name="l1ps", bufs=4, space="PSUM") as l1ps,
                tc.tile_pool(name="l2ps", bufs=4, space="PSUM") as l2ps,
            ):
                nc.sync.dma_start(out=h2[24][64:128, :], in_=zeros64.ap()[:])
                # PE warmup: keep the array busy during the initial DMA
                # ramp so HAM un-throttles before real matmuls arrive.
                # Dummy MMs over the (already zeroed) h2[24] hi half; the
                # consumed psum bank is start=True-cleared by later users.
                wps = l1ps.tile([128, 512], f32, tag="l1", name="warm_ps")
                zsrc = h2[24][64:96, 0:512]
                for wi in range(14):
                    nc.tensor.matmul(wps[:], zsrc[:, 0:128], zsrc[:, :],
                                     start=True, stop=True)
                out1 = [[None] * 8 for _ in range(16)]

                def emit_l1_row(r):
                    w1row = w1_pool.tile([128, 256], f32r, tag="w1",
                                         name=f"w1_{r}")
                    w1src = w1t.ap()[r].rearrange("p g c -> p (g c)")
                    if r == 0:
                        for i in range(4):
                            nc.sync.dma_start(out=w1row[32*i:32*i+32, :],
                                              in_=w1src[32*i:32*i+32, :])
                    else:
                        nc.sync.dma_start(out=w1row[:], in_=w1src)
                    for g in range(2):
                        xt = xpp_pool.tile([128, BS], f32r, tag="xp",
                                           name=f"xp_{r}_{g}")
                        if r == 0:
                            for i in range(4):
                                nc.sync.dma_start(
                                    out=xt[32*i:32*i+32, :],
                                    in_=x_pp.ap()[r, g][32*i:32*i+32, :])
                        else:
                            nc.sync.dma_start(out=xt[:], in_=x_pp.ap()[r, g])
                        for i in range(4):
                            ps = l1ps.tile([128, 512], f32, tag="l1",
                                           name=f"l1ps_{r}_{g}_{i}")
                            nc.tensor.matmul(
                                ps[:],
                                rc(w1row[32*i:32*i+32, 128*g:128*g+128]),
                                rc(xt[32*i:32*i+32, :]),
                                start=True, stop=True,
                                tile_position=(32 * i, 0))
                            ot = o1_pool.tile([128, 512], f16, tag="o1",
                                              name=f"o1_{r}_{4*g+i}")
                            relu_evac(ot[:], ps[:])
                            out1[r][4 * g + i] = ot

                def load_w2(pos):
                    w2til = w2_pool.tile([128, 512], f16, tag="w2",
                                         name=f"w2_{pos}")
                    nc.sync.dma_start(out=w2til[:], in_=w2t.ap()[pos])
                    return w2til

                def emit_l2_pair(T, hA, wA, hB, wB):
                    # Two positions concurrently on PE col strips 0-63 /
                    # 64-127 (tile_position col tiling), each chain
                    # accumulating in its own PSUM bank so the start=True
                    # bank clears stay independent of scheduler order.
                    wtA = load_w2(hA * 7 + wA)
                    wtB = None if hB is None else load_w2(hB * 7 + wB)
                    psA = l2ps.tile([128, 512], f32, tag="l2",
                                    name=f"l2psA_{T}")
                    psB = None
                    if wtB is not None:
                        psB = l2ps.tile([128, 512], f32, tag="l2",
                                        name=f"l2psB_{T}")
                    for kt in range(8):
                        kh, t = divmod(kt, 2)
                        nc.tensor.matmul(
                            psA[0:64, :],
                            wtA[:, 64*kt:64*kt+64],
                            out1[2*hA+kh][wA+t][:],
                            start=(kt == 0), stop=(kt == 7),
                            tile_position=(0, 0))
                        if wtB is not None:
                            nc.tensor.matmul(
                                psB[64:128, :],
                                wtB[:, 64*kt:64*kt+64],
                                out1[2*hB+kh][wB+t][:],
                                start=(kt == 0), stop=(kt == 7),
                                tile_position=(0, 64))
                    relu_evac(h2[T][0:64, :], psA[0:64, :])
                    if wtB is not None:
                        relu_evac(h2[T][64:128, :], psB[64:128, :])

                def emit_l2_pass(h):
                    for j in range(3):
                        emit_l2_pair(h * 3 + j, h, 2 * j, h, 2 * j + 1)
                    # cross pairs (w=6, rows h-2 & h-1) are deferred one
                    # pass: their hi-chain rhs tiles are the last evacs of
                    # row 2h-1, and the in-order PE would stall
                    # head-of-line waiting for them if emitted in pass h-1.
                    if h >= 2 and h % 2 == 0:
                        pi = (h - 2) // 2
                        emit_l2_pair(21 + pi, h - 2, 6, h - 1, 6)
                    if h == 6:
                        # Re-emitting T=23 here is intentional: it writes
                        # identical data a second time, but the extra pair
                        # keeps the PE stream dense across the last L2 pass
                        # and measures consistently faster.
                        emit_l2_pair(23, 4, 6, 5, 6)
                        emit_l2_pair(24, 6, 6, None, None)

                for r in range(16):
                    emit_l1_row(r)
                    if r == 1:
                        # second keep-warm burst: l2ps banks are idle until
                        # the first L2 pass; fills the DMA-paced early rows
                        # so HAM stays un-throttled.
                        wps2 = l2ps.tile([128, 512], f32, tag="l2",
                                         name="warm_ps2")
                        for wi in range(10):
                            nc.tensor.matmul(wps2[:], zsrc[:, 0:128],
                                             zsrc[:, :],
                                             start=True, stop=True)
                    if r >= 3 and r % 2 == 1:
                        emit_l2_pass((r - 3) // 2)

            # --------------- phase 2: FC head ---------------
            with (
                tc.tile_pool(name="fcio", bufs=12) as fcio_pool,
                tc.tile_pool(name="fcps", bufs=2, space="PSUM") as fcps,
                tc.tile_pool(name="fc3ps", bufs=2, space="PSUM") as fc3ps,
            ):
                h3 = []
                for m in range(8):
                    wt = fcw_pool.tile([128, 25 * 128], f32r, tag="fc1w",
                                       name=f"fc1w_{m}")
                    src = fc1m.ap()[m].rearrange("p k c -> p (k c)")
                    nc.sync.dma_start(out=wt[:, 0:1600], in_=src[:, 0:1600])
                    nc.sync.dma_start(out=wt[:, 1600:3200],
                                      in_=src[:, 1600:3200])
                    ps = fcps.tile([128, 512], f32, tag="fc",
                                   name=f"fc1ps_{m}")
                    for k in range(25):
                        nc.tensor.matmul(ps[:],
                                         rc(wt[:, 128*k:128*k+128]),
                                         rc(h2[k][:]),
                                         start=(k == 0), stop=(k == 24))
                    ot = fcio_pool.tile([128, 512], f32r, tag="h3",
                                        name=f"h3_{m}", bufs=8)
                    relu_evac(ot[:], ps[:])
                    h3.append(ot)
                h4 = []
                for m in range(4):
                    wt = fcw_pool.tile([128, 8 * 128], f32r, tag="fc2w",
                                       name=f"fc2w_{m}")
                    nc.sync.dma_start(
                        out=wt[:],
                        in_=fc2t.ap()[m].rearrange("p k c -> p (k c)"))
                    ps = fcps.tile([128, 512], f32, tag="fc",
                                   name=f"fc2ps_{m}")
                    for k in range(8):
                        nc.tensor.matmul(ps[:],
                                         rc(wt[:, 128*k:128*k+128]),
                                         rc(h3[k][:]),
                                         start=(k == 0), stop=(k == 7))
                    ot = fcio_pool.tile([128, 512], f32r, tag="h4",
                                        name=f"h4_{m}", bufs=4)
                    relu_evac(ot[:], ps[:])
                    h4.append(ot)
                w3 = fcio_pool.tile([128, 40], f32r, tag="fc3w",
                                    name="fc3w", bufs=1)
                nc.sync.dma_start(
                    out=w3[:], in_=fc3t.ap().rearrange("p k o -> p (k o)"))
                for b4 in range(4):
                    ps = fc3ps.tile([128, 10], f32, tag="fc3",
                                    name=f"fc3ps_{b4}")
                    for k in range(4):
                        nc.tensor.matmul(
                            ps[:],
                            rc(h4[k][:, 128*b4:128*b4+128]),
                            rc(w3[:, 10*k:10*k+10]),
                            start=(k == 0), stop=(k == 3))
                    ot = fcio_pool.tile([128, 10], f32, tag="yout",
                                        name=f"y_{b4}", bufs=4)
                    nc.vector.tensor_copy(ot[:], ps[:])
                    nc.sync.dma_start(out=y.ap()[128*b4:128*b4+128, :],
                                      in_=ot[:])
    nc.compile()
    return nc


def kernel(x, conv1w, conv2w, fc1, fc2, fc3):
    global LAST_EXEC_NS
    from concourse.bass_utils import run_bass_kernel_spmd

    x = np.ascontiguousarray(np.asarray(x, dtype=np.float32))
    conv1w = np.ascontiguousarray(np.asarray(conv1w, dtype=np.float32))
    conv2w = np.ascontiguousarray(np.asarray(conv2w, dtype=np.float32))
    fc1 = np.ascontiguousarray(np.asarray(fc1, dtype=np.float32))
    fc2 = np.ascontiguousarray(np.asarray(fc2, dtype=np.float32))
    fc3 = np.ascontiguousarray(np.asarray(fc3, dtype=np.float32))

    if not _NC_CACHE:
        _NC_CACHE.append(_build_nc())
    nc = _NC_CACHE[0]

    xpp = _prep_x(x)
    shared = {
        "zeros64": np.zeros((64, 512), np.float32),
        "w1t": _prep_w1(conv1w),
        "w2t": _prep_w2(conv2w),
        "fc1m": _prep_fc1(fc1),
        "fc2t": _prep_fc2(fc2),
        "fc3t": _prep_fc3(fc3),
    }
    in_maps = [{**shared, "x_pp": xpp[c]} for c in range(N_CORES)]
    res = run_bass_kernel_spmd(nc, in_maps, list(range(N_CORES)))
    LAST_EXEC_NS = res.exec_time_ns
    return np.concatenate([r["y"] for r in res.results], axis=0)
